# revision 34
# baseline (speedup 1.0000x reference)
"""Trainium2 Bass kernel for nn_Encoder_Postnet (B=16, T=8192, TP=512, E=256).

Exact algebra:
    idx  = aligner_indices(align_phone, text_phone)     # host scan (sequential int walk)
    out  = enc2[b, idx] + PEW[t] + pitch[b,t]*Wp + beats[b,t]*EBd
where
    enc2 = encoder_out @ (I + W_pos)                    # device PE, f32
    PEW  = pe @ W_pos + b_pos + b_pitch + emb_beats[0]  # host constant table
    Wp   = W_pitch[0],  EBd = emb_beats[1] - emb_beats[0]

Sharding: pure data parallel, 2 batches per core across 8 cores.

The frame gather enc2[idx] exploits idx monotonicity: a 128-frame group's rows
span few enc2 rows, so the gather becomes one matmul per group on the
(otherwise idle) TensorE against a 64-aligned "window" of enc2: the device
keeps enc2win[k] = [enc2 rows 64k..64k+126; Wp; EBd] (128 partitions), and the
host builds lhsT columns: rows 0-125 one-hot selecting the window row per
frame, rows 126-127 carrying pitch/beats so the rank-2 aux term rides in the
SAME matmul. PSUM accumulates; one DVE tensor_tensor adds the resident PEW
table and writes fp16 output. Arbitrary idx is handled by covering a group
with multiple windows (extra accumulating matmuls); SPMD uniformity by taking
the cross-core union of window entries (unused entries have all-zero lhsT
columns).
"""
import numpy as np

import concourse.bacc as bacc
import concourse.bass as bass
import concourse.mybir as mybir
import concourse.tile as tile
from concourse.bass_utils import run_bass_kernel_spmd

# ---- problem constants (hardcoded per harness contract) ----
B, T, TP, E = 16, 8192, 512, 256
NCORES = 8
BPC = B // NCORES            # batches per core = 2
ROWS = BPC * TP              # enc2 rows per core = 1024
NBLK = ROWS // 128           # 128-row blocks of enc2 = 8
WALIGN = 64                  # window alignment
WROWS = 126                  # usable enc2 rows per window (126/127 = Wp/EBd)
KWIN = 128                   # matmul contraction per window entry (FWL needs 128)
NWIN = ROWS // WALIGN        # 64-aligned windows = 16
CH = 1024                    # frames per chunk
NCH = T // CH                # chunks per batch = 8
NG = CH // 128               # 128-frame groups per chunk = 8
NCHUNK = BPC * NCH           # chunks per core = 16

F32 = mybir.dt.float32
FP16 = mybir.dt.float16

_PROGRAM_CACHE: dict = {}


# ---------------- host-side pieces ----------------

def aligner_idx_host(align_phone: np.ndarray, text_phone: np.ndarray) -> np.ndarray:
    """Exact numpy equivalent of the reference aligner_indices scan."""
    b, t = align_phone.shape
    tp_last = text_phone.shape[1] - 1
    idx = np.zeros((b, t), dtype=np.int32)
    ind = np.zeros(b, dtype=np.int32)
    before = text_phone[:, 0].copy()
    barange = np.arange(b)
    for j in range(1, t):
        a = align_phone[:, j]
        same = a == before
        ind = np.minimum(np.where(same, ind, ind + 1), tp_last)
        before = np.where(same, before, text_phone[barange, ind])
        idx[:, j] = ind
    return idx


def sinusoid_pe_host(length, dim):
    pos = np.arange(length, dtype=np.float32)[:, None]
    div = np.exp(np.arange(0, dim, 2, dtype=np.float32) * (-(np.log(10000.0) / dim)))
    ang = pos * div
    pe = np.zeros((length, dim), np.float32)
    pe[:, 0::2] = np.sin(ang)
    pe[:, 1::2] = np.cos(ang)
    return pe


def windows_for_group(gi: np.ndarray) -> list:
    """Minimal aligned windows covering the rows in gi (sorted)."""
    rows = np.unique(gi)
    wins = []
    i = 0
    while i < len(rows):
        k = int(rows[i]) // WALIGN
        wins.append(k)
        top = WALIGN * k + WROWS
        while i < len(rows) and rows[i] < top:
            i += 1
    return wins


def group_windows(idx_rows: np.ndarray):
    """per chunk per group: list of window ids for this core."""
    out = []
    for lb in range(BPC):
        for c in range(NCH):
            chunk = []
            for g in range(NG):
                f0 = c * CH + g * 128
                chunk.append(windows_for_group(idx_rows[lb, f0:f0 + 128]))
            out.append(chunk)
    return out


# ---------------- device program ----------------

def build_program(canon_plan, ncols_total) -> bass.Bass:
    """canon_plan[ci][g] = list of (coloff, win_k)."""
    nc = bacc.Bacc("TRN2", num_devices=NCORES, debug=False, enable_asserts=False)

    enc = nc.dram_tensor("enc", [ROWS, E], F32, kind="ExternalInput")
    w2 = nc.dram_tensor("w2", [E, E], F32, kind="ExternalInput")
    ident = nc.dram_tensor("ident", [128, 128], F32, kind="ExternalInput")
    w3rep = nc.dram_tensor("w3rep", [2, NWIN, E], FP16, kind="ExternalInput")
    pew = nc.dram_tensor("pew", [T, E], FP16, kind="ExternalInput")
    oh = nc.dram_tensor("oh", [KWIN, ncols_total], FP16, kind="ExternalInput")
    out = nc.dram_tensor("out", [BPC * T, E], FP16, kind="ExternalOutput")

    with tile.TileContext(nc) as tc:
        with (
            tc.tile_pool(name="const", bufs=1) as cpool,
            tc.tile_pool(name="outp", bufs=3) as opool,
        ):
            # ---- loads: enc first (it gates the PE prologue chain), then the
            # streaming tables in per-chunk slices so chunk 0 starts early ----
            enc_sb = cpool.tile([128, NBLK, E], F32, tag="enc")
            nc.sync.dma_start(enc_sb[:], enc.ap().rearrange("(r p) e -> p r e", p=128))
            w2_sb = cpool.tile([128, 2, E], F32, tag="w2")
            nc.scalar.dma_start(w2_sb[:], w2.ap().rearrange("(k p) e -> p k e", p=128))
            ident_sb = cpool.tile([128, 128], F32, tag="ident")
            nc.scalar.dma_start(ident_sb[:], ident.ap())

            # stream oh + pew on sync in consumption order (scalar's FIFO is
            # busy with the windows build, which waits on the enc2 chain)
            oh_sb = cpool.tile([KWIN, ncols_total], FP16, tag="oh")
            oh_bounds = []
            for ci in range(NCHUNK):
                lo = canon_plan[ci][0][0][0]
                hi = canon_plan[ci][-1][-1][0] + 128
                oh_bounds.append((lo, hi))
            pew_sb = cpool.tile([128, T // 128, E], FP16, tag="pew")
            pew_ap = pew.ap().rearrange("(c p) e -> p c e", p=128)
            for ci in range(NCHUNK):
                lo, hi = oh_bounds[ci]
                nc.sync.dma_start(oh_sb[:, lo:hi], oh.ap()[:, lo:hi])
                if ci < NCH:
                    nc.sync.dma_start(
                        pew_sb[:, ci * NG:(ci + 1) * NG, :],
                        pew_ap[:, ci * NG:(ci + 1) * NG, :],
                    )
            encT_sb = cpool.tile([128, 2 * NBLK, 128], F32, tag="encT")
            enc2_sb = cpool.tile([128, NBLK, E], FP16, tag="enc2")
            win_sb = cpool.tile([KWIN, NWIN, E], FP16, tag="win")
            # only window NWIN-1's upper piece has no source rows; zero it so
            # 0-weight matmul columns can't touch NaN garbage
            nc.vector.memset(win_sb[64:WROWS, NWIN - 1, :], 0.0)
            with tc.tile_pool(name="psum_pro", bufs=4, space="PSUM") as ppro:
                for rt in range(NBLK):
                    for k in range(2):
                        pt = ppro.tile([128, 128], F32, tag="ptr")
                        nc.tensor.transpose(
                            out=pt[:],
                            in_=enc_sb[:, rt, k * 128:(k + 1) * 128],
                            identity=ident_sb[:],
                        )
                        nc.vector.tensor_copy(
                            out=encT_sb[:, k * NBLK + rt, :], in_=pt[:]
                        )
                for rt in range(NBLK):
                    pe2 = ppro.tile([128, E], F32, tag="pe2")
                    nc.tensor.matmul(
                        out=pe2[:], lhsT=encT_sb[:, rt, :], rhs=w2_sb[:, 0, :],
                        start=True, stop=False,
                    )
                    nc.tensor.matmul(
                        out=pe2[:], lhsT=encT_sb[:, NBLK + rt, :],
                        rhs=w2_sb[:, 1, :], start=False, stop=True,
                    )
                    nc.vector.tensor_copy(out=enc2_sb[:, rt, :], in_=pe2[:])
            # windows via 4 strided DMAs (k parity classes):
            # even k: rows [128(k/2), +126) = aligned block copy
            nc.scalar.dma_start(win_sb[0:126, 0:NWIN:2, :], enc2_sb[0:126, :, :])
            # odd k piece 1: rows [64k, 64k+64) = block k//2 partitions 64..128
            nc.scalar.dma_start(win_sb[0:64, 1:NWIN:2, :], enc2_sb[64:128, :, :])
            # odd k piece 2: rows [64k+64, 64k+126) = block k//2+1 partitions 0..62
            nc.scalar.dma_start(
                win_sb[64:126, 1:NWIN - 2:2, :], enc2_sb[0:62, 1:NBLK, :]
            )
            # constant rows 126/127 = Wp, EBd for every window
            nc.scalar.dma_start(win_sb[126:128, :, :], w3rep.ap())

            # ---- main loop ----
            with tc.tile_pool(name="psum_main", bufs=2, space="PSUM") as pmain:
                for ci in range(NCHUNK):
                    lb, c = divmod(ci, NCH)
                    ps = pmain.tile([128, NG, E], F32, tag="ps")
                    for g in range(NG):
                        entries = canon_plan[ci][g]
                        n = len(entries)
                        for j, (coloff, k) in enumerate(entries):
                            nc.tensor.matmul(
                                out=ps[:, g, :],
                                lhsT=oh_sb[:, coloff:coloff + 128],
                                rhs=win_sb[:, k, :],
                                start=(j == 0), stop=(j == n - 1),
                            )
                    # evacuation split: half the chunks borrow ScalarE for the
                    # PSUM copy (then a cheap all-fp16 2x DVE add); the rest do
                    # the direct 1x DVE psum+pew add — balances DVE vs ScalarE
                    o = opool.tile([128, NG, E], FP16, tag="o")
                    if ci % 2 == 0:
                        tmp = opool.tile([128, NG, E], FP16, tag="tmp")
                        nc.scalar.copy(out=tmp[:], in_=ps[:])
                        nc.vector.tensor_tensor(
                            out=o[:], in0=tmp[:],
                            in1=pew_sb[:, c * NG:(c + 1) * NG, :],
                            op=mybir.AluOpType.add,
                        )
                    else:
                        nc.vector.tensor_tensor(
                            out=o[:], in0=ps[:],
                            in1=pew_sb[:, c * NG:(c + 1) * NG, :],
                            op=mybir.AluOpType.add,
                        )
                    base = lb * T + c * CH
                    out_eng = nc.sync if ci % 2 == 0 else nc.scalar
                    out_eng.dma_start(
                        out.ap()[base:base + CH, :].rearrange(
                            "(cc p) e -> p cc e", p=128
                        ),
                        o[:],
                    )
    nc.compile()
    return nc


# ---------------- host orchestration ----------------

def make_in_maps(encoder_out, align_phone, text_phone, pitch, beats,
                 W_pitch, b_pitch, W_pos, b_pos, emb_beats):
    idx = aligner_idx_host(np.asarray(align_phone), np.asarray(text_phone))  # [B, T]

    pe = sinusoid_pe_host(T, E)
    pew = (pe @ np.asarray(W_pos) + np.asarray(b_pos) + np.asarray(b_pitch)
           + np.asarray(emb_beats)[0]).astype(np.float32)
    w2 = (np.eye(E, dtype=np.float32) + np.asarray(W_pos)).astype(np.float32)
    ident = np.eye(128, dtype=np.float32)
    wp = np.asarray(W_pitch)[0].astype(np.float32)
    ebd = (np.asarray(emb_beats)[1] - np.asarray(emb_beats)[0]).astype(np.float32)
    w3 = np.stack([wp, ebd]).astype(np.float16)  # [2, E]
    w3rep = np.broadcast_to(w3[:, None, :], (2, NWIN, E)).copy()

    enc = np.ascontiguousarray(np.asarray(encoder_out), dtype=np.float32)  # [B, TP, E]
    pitch2 = np.asarray(pitch)[:, :, 0].astype(np.float32)
    beats2 = np.asarray(beats)[:, :, 0].astype(np.float32)

    idx_rows_all = []
    wins_all = []
    for core in range(NCORES):
        bs = slice(core * BPC, (core + 1) * BPC)
        idx_rows = idx[bs] + (np.arange(BPC)[:, None] * TP)
        idx_rows_all.append(idx_rows)
        wins_all.append(group_windows(idx_rows))

    # canonical plan: per (chunk, group) union of window ids across cores
    canon_plan = []
    off = 0
    for ci in range(NCHUNK):
        chunk_plan = []
        for g in range(NG):
            ks = sorted({k for core in range(NCORES) for k in wins_all[core][ci][g]})
            entries = []
            for k in ks:
                entries.append((off, k))
                off += 128
            chunk_plan.append(entries)
        canon_plan.append(chunk_plan)
    ncols_total = off

    per_core = []
    m = np.arange(128)
    for core in range(NCORES):
        bs = slice(core * BPC, (core + 1) * BPC)
        idx_rows = idx_rows_all[core]
        onehot = np.zeros((KWIN, ncols_total), dtype=np.float16)
        for ci in range(NCHUNK):
            lb, c = divmod(ci, NCH)
            for g in range(NG):
                f0 = c * CH + g * 128
                gi = idx_rows[lb, f0:f0 + 128]
                my_wins = wins_all[core][ci][g]
                entries = canon_plan[ci][g]
                # row -> my window (first of my windows covering it)
                assigned = np.full(128, -1, dtype=np.int64)
                for k in my_wins:
                    in_win = ((gi >= WALIGN * k) & (gi < WALIGN * k + WROWS)
                              & (assigned < 0))
                    assigned[in_win] = k
                aux_done = False
                for (coloff, k) in entries:
                    if k not in my_wins:
                        continue
                    sel = assigned == k
                    onehot[gi[sel] - WALIGN * k, coloff + m[sel]] = 1.0
                    if not aux_done:
                        fr = slice(c * CH + g * 128, c * CH + g * 128 + 128)
                        onehot[WROWS, coloff:coloff + 128] = pitch2[core * BPC + lb, fr]
                        onehot[WROWS + 1, coloff:coloff + 128] = beats2[
                            core * BPC + lb, fr]
                        aux_done = True
        per_core.append({
            "enc": enc[bs].reshape(ROWS, E),
            "w2": w2,
            "ident": ident,
            "w3rep": w3rep,
            "pew": pew.astype(np.float16),
            "oh": onehot,
        })

    return per_core, canon_plan, ncols_total


def get_program(canon_plan, ncols_total) -> bass.Bass:
    key = (tuple(tuple(tuple(e) for e in cg) for cg in canon_plan), ncols_total)
    if key not in _PROGRAM_CACHE:
        _PROGRAM_CACHE[key] = build_program(canon_plan, ncols_total)
    return _PROGRAM_CACHE[key]


def kernel(**inputs) -> np.ndarray:
    in_maps, canon_plan, ncols_total = make_in_maps(**inputs)
    nc = get_program(canon_plan, ncols_total)
    res = run_bass_kernel_spmd(nc, in_maps, core_ids=list(range(NCORES)))
    outs = [r["out"].astype(np.float32).reshape(BPC, T, E) for r in res.results]
    return np.concatenate(outs, axis=0)
